# revision 1
# baseline (speedup 1.0000x reference)
"""Grouped Conv2d (512 groups, 2->2 ch/group, 3x3 VALID) on 8 trn2 NeuronCores.

Strategy:
  - Shard the 512 groups across 8 cores: 64 groups = 128 channels per core,
    which exactly fills the 128 SBUF partitions. Fully independent (no
    collectives); batch stays whole on every core.
  - On-device compute: for each 3x3 tap (kh,kw) build a 128x128
    block-diagonal weight matrix (64 blocks of 2x2) host-side; the grouped
    conv then becomes 9 accumulating PE matmuls per output tile:
        psum[oc, i, j] += W_tap[ic, oc]^T . x[ic, i+kh, j+kw]
    run in float32r (full-rate fp32 on the PE for moving dim >= 256).
  - Output rows are produced in 6 chunks of 9 rows (9*54 = 486 <= 512 fp32
    = one PSUM bank), evicted PSUM->SBUF on alternating scalar/vector
    engines, and DMA'd back per batch.
"""

import sys

import numpy as np

for _p in ("/opt/trn_rl_repo",):
    if _p not in sys.path:
        sys.path.insert(0, _p)

import concourse.bacc as bacc
import concourse.bass as bass
import concourse.tile as tile
from concourse import mybir
from concourse.bass_utils import run_bass_kernel_spmd

N_CORES = 8
B, C, H, W = 16, 1024, 56, 56
KH = KW = 3
HO, WO = H - KH + 1, W - KW + 1  # 54, 54
CPC = C // N_CORES  # 128 channels (64 groups) per core
ROWS_PER_CHUNK = 9  # 9*54 = 486 fp32 <= 512 (one PSUM bank)
N_CHUNKS = HO // ROWS_PER_CHUNK  # 6

_NC_CACHE = {}


def _build_program(repeats=1):
    nc = bacc.Bacc(
        "TRN2", target_bir_lowering=False, debug=False, num_devices=N_CORES
    )
    f32 = mybir.dt.float32
    f32r = mybir.dt.float32r

    x_d = nc.declare_dram_parameter("x", [B, CPC, H, W], f32, isOutput=False)
    wm_d = nc.declare_dram_parameter(
        "wm", [CPC, KH * KW, CPC], f32, isOutput=False
    )
    y_d = nc.declare_dram_parameter("y", [B, CPC, HO, WO], f32, isOutput=True)

    with tile.TileContext(nc) as tc:
        with (
            tc.tile_pool(name="wpool", bufs=1) as wpool,
            tc.tile_pool(name="xpool", bufs=4) as xpool,
            tc.tile_pool(name="opool", bufs=3) as opool,
            tc.tile_pool(name="psum", bufs=7, space="PSUM") as ppool,
            tc.tile_pool(name="scratch", bufs=1, space="PSUM") as spool,
        ):
            wt = wpool.tile([CPC, KH * KW, CPC], f32r)
            nc.sync.dma_start(out=wt[:], in_=wm_d[:].bitcast(f32r))

            # The fused f32r matmul (LDW+MM) supports only ONE semaphore
            # wait; Tile would otherwise put {wt-DMA, xt-DMA} (2 waits) on
            # the first matmul of each batch. These "sync" matmuls absorb
            # the DMA waits; PE program order covers the rest.
            scr = spool.tile([CPC, 512], f32)
            nc.tensor.matmul(
                scr[:, :2], lhsT=wt[:, 0, :], rhs=wt[:, 0, :2],
                start=True, stop=True,
            )
            # Dummy matmuls keep PE busy during the initial x DMA fill so
            # the HAM clock gate ramps to 2.4 GHz before real work arrives.
            for _ in range(16):
                nc.tensor.matmul(
                    scr[:, :256], lhsT=wt[:, 0, :], rhs=wt[:, 0:2, :],
                    start=True, stop=True,
                )

            def body():
                _emit_batches(nc, tc, xpool, opool, ppool, x_d, y_d, wt, scr)

            if repeats == 1:
                body()
            else:
                with tc.For_i(0, repeats):
                    body()
    nc.compile()
    return nc


def _emit_batches(nc, tc, xpool, opool, ppool, x_d, y_d, wt, scr):
    f32 = mybir.dt.float32
    f32r = mybir.dt.float32r
    HSPLIT = 30  # rows 0..29 cover chunks 0-2; rows 30..55 cover chunks 3-5
    OSPLIT = 3 * ROWS_PER_CHUNK  # first 3 output chunks ship early
    for n in range(B):
                xt = xpool.tile([CPC, H, W], f32r)
                nc.sync.dma_start(
                    out=xt[:, :HSPLIT, :], in_=x_d[n, :, :HSPLIT, :].bitcast(f32r)
                )
                nc.sync.dma_start(
                    out=xt[:, HSPLIT:, :], in_=x_d[n, :, HSPLIT:, :].bitcast(f32r)
                )
                nc.tensor.matmul(
                    scr[:, :2], lhsT=wt[:, 0, :], rhs=xt[:, 0, :2],
                    start=True, stop=True,
                )
                nc.tensor.matmul(
                    scr[:, :2], lhsT=wt[:, 0, :], rhs=xt[:, H - 1, :2],
                    start=True, stop=True,
                )
                ot = opool.tile([CPC, HO, WO], f32)
                for c in range(N_CHUNKS):
                    r0 = c * ROWS_PER_CHUNK
                    pt = ppool.tile([CPC, ROWS_PER_CHUNK, WO], f32)
                    t = 0
                    for kh in range(KH):
                        for kw in range(KW):
                            nc.tensor.matmul(
                                pt[:],
                                lhsT=wt[:, t, :],
                                rhs=xt[
                                    :,
                                    r0 + kh : r0 + kh + ROWS_PER_CHUNK,
                                    kw : kw + WO,
                                ],
                                start=(t == 0),
                                stop=(t == KH * KW - 1),
                            )
                            t += 1
                    dst = ot[:, r0 : r0 + ROWS_PER_CHUNK, :]
                    if c % 2 == 0:
                        nc.scalar.activation(
                            dst, pt[:], mybir.ActivationFunctionType.Copy
                        )
                    else:
                        nc.vector.tensor_copy(dst, pt[:])
                    if c == 2:
                        nc.sync.dma_start(
                            out=y_d[n, :, :OSPLIT, :], in_=ot[:, :OSPLIT, :]
                        )
                nc.sync.dma_start(
                    out=y_d[n, :, OSPLIT:, :], in_=ot[:, OSPLIT:, :]
                )


def _get_nc(repeats=1):
    if repeats not in _NC_CACHE:
        _NC_CACHE[repeats] = _build_program(repeats)
    return _NC_CACHE[repeats]


def _make_wmats(w):
    """Per-core lhsT weight mats, shape (128, 9, 128): wm[ic, t, oc]."""
    oc = np.arange(CPC)
    mats = []
    for cid in range(N_CORES):
        ws = np.asarray(w[cid * CPC : (cid + 1) * CPC], dtype=np.float32)
        wm = np.zeros((CPC, KH * KW, CPC), dtype=np.float32)
        for icg in range(2):
            ic = (oc // 2) * 2 + icg
            # advanced indexing on dims 0 and 2 -> result dims (pair, tap)
            wm[ic, :, oc] = ws[oc, icg].reshape(CPC, KH * KW)
        mats.append(wm)
    return mats


def _run(x, w, trace=False, **kwargs):
    nc = _get_nc()
    x = np.asarray(x, dtype=np.float32)
    wmats = _make_wmats(w)
    in_maps = [
        {
            "x": np.ascontiguousarray(x[:, cid * CPC : (cid + 1) * CPC]),
            "wm": wmats[cid],
        }
        for cid in range(N_CORES)
    ]
    res = run_bass_kernel_spmd(
        nc, in_maps, list(range(N_CORES)), trace=trace, **kwargs
    )
    y = np.concatenate(
        [res.results[i]["y"] for i in range(N_CORES)], axis=1
    )
    return y, res


def kernel(x, w):
    y, _ = _run(x, w, trace=False)
    return y



# revision 2
# speedup vs baseline: 1.8285x; 1.8285x over previous
"""Grouped Conv2d (512 groups, 2->2 ch/group, 3x3 VALID) on 8 trn2 cores.

Strategy (v3): 64x64 PE-array tiling (tile_position, 2x2 = 4 concurrent
tiles) + bf16 I/O.

  - Shard the 512 groups across 8 cores: 64 groups = 128 channels per
    core; batch stays whole on every core; no collectives.
  - The grouped conv's weight is block-diagonal with only 2 useful
    contraction channels per output, so a full 128x128 matmul wastes the
    PE array.  Instead run 4 concurrent 64x64 tile matmuls: tile (i,j)
    convolves channel-half j (64 ch = 32 groups) of batch 2r+i, reading
    rhs from SBUF partitions 64i and accumulating into PSUM partitions
    64j, bank 2i + chunk-parity.
  - Why 64x64 and not 32x32: the Tile framework emits a semaphore
    increment on every matmul (~26 ns, serialized) and walrus emits one
    LDWEIGHTS per matmul (~27 ns, serialized), so a 32x32 schedule
    (3456 MMs) is instruction-overhead-bound at ~39 ns/MM.  At 64x64
    the MM count is 1728 and the 4-way stream rate is 50.6 ns/MM, which
    hides the per-MM overheads.
  - Work unit: batch-pair r x chunk-pair p (18 output rows): 9 taps x
    2 chunks x 4 tiles = 72 matmuls (N = 9*54 = 486 <= one PSUM bank)
    into 4 banks; bank 2i+c holds (batch 2r+i, chunk 2p+c) as
    [128ch, 9, 54].  ppool bufs=2 double-buffers the bank set so
    evictions overlap the next superblock's matmuls.
  - PSUM fp32 accumulation; evictions alternate scalar/vector engines,
    converting to bf16; DMA out per batch/chunk-pair.
  - bf16 inputs/weights/outputs halve the HBM traffic (memory-regime
    roofline); the host casts x to bf16 and the bf16 output back to
    fp32.  End-to-end rel err ~2.7e-3 (bf16-limited).
"""

import sys

import numpy as np

for _p in ("/opt/trn_rl_repo",):
    if _p not in sys.path:
        sys.path.insert(0, _p)

import ml_dtypes

import concourse.bacc as bacc
import concourse.tile as tile
from concourse import mybir
from concourse.bass_utils import run_bass_kernel_spmd

BF16 = ml_dtypes.bfloat16

N_CORES = 8
B, C, H, W = 16, 1024, 56, 56
KH = KW = 3
HO, WO = H - KH + 1, W - KW + 1  # 54, 54
CPC = C // N_CORES  # 128 channels (64 groups) per core
RPC = 9  # rows per chunk: 9*54 = 486 fp32 <= one PSUM bank
N_CHUNKS = HO // RPC  # 6
N_GROUPS = B // 2  # batch-pairs
N_PAIRS = N_CHUNKS // 2  # chunk-pairs of 18 rows

_NC_CACHE = {}


def _build_program(repeats=1):
    nc = bacc.Bacc(
        "TRN2", target_bir_lowering=False, debug=False, num_devices=N_CORES
    )
    f32 = mybir.dt.float32
    bf16 = mybir.dt.bfloat16

    x_d = nc.declare_dram_parameter("x", [B, CPC, H, W], bf16, isOutput=False)
    wm_d = nc.declare_dram_parameter(
        "wm", [CPC, 2, KH * KW, 64], bf16, isOutput=False
    )
    y_d = nc.declare_dram_parameter("y", [B, CPC, HO, WO], bf16, isOutput=True)

    with tile.TileContext(nc) as tc:
        with (
            tc.tile_pool(name="wpool", bufs=1) as wpool,
            tc.tile_pool(name="xpool", bufs=3) as xpool,
            tc.tile_pool(name="opool", bufs=3) as opool,
            tc.tile_pool(name="ppool", bufs=2, space="PSUM") as ppool,
        ):
            wt = wpool.tile([CPC, 2, KH * KW, 64], bf16)
            nc.sync.dma_start(out=wt[:], in_=wm_d[:])

            # HAM clock-gate warmup on a throwaway psum generation while
            # the first x DMA is in flight.
            pts_w = [
                ppool.tile([CPC, RPC, WO], f32, name=f"pt{b}")
                for b in range(4)
            ]
            for rep in range(8):
                for k in range(4):
                    i, j = k // 2, k % 2
                    nc.tensor.matmul(
                        pts_w[k][64 * j : 64 * j + 64, 0, 0:32],
                        lhsT=wt[64 * i : 64 * i + 64, 0, 0, :],
                        rhs=wt[64 * i : 64 * i + 64, 0, 0:1, 0:32],
                        start=True,
                        stop=True,
                        tile_position=(64 * i, 64 * j),
                    )

            def body():
                _emit_groups(nc, tc, xpool, opool, ppool, x_d, y_d, wt)

            if repeats == 1:
                body()
            else:
                with tc.For_i(0, repeats):
                    body()
    nc.compile()
    return nc


def _emit_groups(nc, tc, xpool, opool, ppool, x_d, y_d, wt):
    f32 = mybir.dt.float32
    bf16 = mybir.dt.bfloat16
    for r in range(N_GROUPS):
        xg = xpool.tile([CPC, 2, H, W], bf16)
        for i in range(2):
            for j in range(2):
                nc.sync.dma_start(
                    out=xg[64 * i : 64 * i + 64, j],
                    in_=x_d[2 * r + i, 64 * j : 64 * j + 64],
                )
        for p in range(N_PAIRS):
            pts = [
                ppool.tile([CPC, RPC, WO], f32, name=f"pt{b}")
                for b in range(4)
            ]
            for t in range(KH * KW):
                kh, kw = t // KW, t % KW
                for c01 in range(2):
                    r0 = (2 * p + c01) * RPC
                    for i in range(2):
                        for j in range(2):
                            nc.tensor.matmul(
                                pts[2 * i + c01][64 * j : 64 * j + 64],
                                lhsT=wt[64 * i : 64 * i + 64, j, t, :],
                                rhs=xg[
                                    64 * i : 64 * i + 64,
                                    j,
                                    r0 + kh : r0 + kh + RPC,
                                    kw : kw + WO,
                                ],
                                start=(t == 0),
                                stop=(t == KH * KW - 1),
                                tile_position=(64 * i, 64 * j),
                            )
            for i in range(2):
                og = opool.tile([CPC, 2 * RPC, WO], bf16, name=f"og{i}")
                for c01 in range(2):
                    src = pts[2 * i + c01][:]
                    dst = og[:, c01 * RPC : (c01 + 1) * RPC, :]
                    if (i + c01) % 2 == 0:
                        nc.scalar.activation(
                            dst, src, mybir.ActivationFunctionType.Copy
                        )
                    else:
                        nc.vector.tensor_copy(dst, src)
                nc.sync.dma_start(
                    out=y_d[
                        2 * r + i, :, 2 * p * RPC : 2 * (p + 1) * RPC, :
                    ],
                    in_=og[:],
                )


def _get_nc(repeats=1):
    if repeats not in _NC_CACHE:
        _NC_CACHE[repeats] = _build_program(repeats)
    return _NC_CACHE[repeats]


def _make_wm(w, cid):
    """Per-core weight tensor [128, 2, 9, 64] bf16: wm[64*i+a, j, t, oc]
    = w[cid*128+64*j+oc, a%2, kh, kw] when a//2 == oc//2 else 0,
    replicated over row-block i (each row-block needs its own copy for
    LDWEIGHTS)."""
    ws = np.asarray(w[cid * CPC : (cid + 1) * CPC], dtype=np.float32)
    wm = np.zeros((64, 2, KH * KW, 64), dtype=np.float32)
    oc = np.arange(64)
    for j in range(2):
        block = ws[64 * j : 64 * j + 64]  # [64 oc, 2 icg, 3, 3]
        for icg in range(2):
            arow = (oc // 2) * 2 + icg
            wm[arow, j, :, oc] = block[oc, icg].reshape(64, KH * KW)
    return np.tile(wm, (2, 1, 1, 1)).astype(BF16)


def _run(x, w, trace=False, **kwargs):
    nc = _get_nc()
    x_bf = np.asarray(x, dtype=np.float32).astype(BF16)
    in_maps = [
        {
            "x": np.ascontiguousarray(
                x_bf[:, cid * CPC : (cid + 1) * CPC]
            ),
            "wm": _make_wm(w, cid),
        }
        for cid in range(N_CORES)
    ]
    res = run_bass_kernel_spmd(
        nc, in_maps, list(range(N_CORES)), trace=trace, **kwargs
    )
    y = np.concatenate(
        [
            res.results[i]["y"].astype(np.float32)
            for i in range(N_CORES)
        ],
        axis=1,
    )
    return y, res


def kernel(x, w):
    y, _ = _run(x, w, trace=False)
    return y


# revision 4
# speedup vs baseline: 2.1045x; 1.1510x over previous
"""Grouped Conv2d (512 groups, 2->2 ch/group, 3x3 VALID) on 8 trn2 cores.

Strategy (v3): 64x64 PE-array tiling (tile_position, 2x2 = 4 concurrent
tiles) + bf16 I/O.

  - Shard the 512 groups across 8 cores: 64 groups = 128 channels per
    core; batch stays whole on every core; no collectives.
  - The grouped conv's weight is block-diagonal with only 2 useful
    contraction channels per output, so a full 128x128 matmul wastes the
    PE array.  Instead run 4 concurrent 64x64 tile matmuls: tile (i,j)
    convolves channel-half j (64 ch = 32 groups) of batch 2r+i, reading
    rhs from SBUF partitions 64i and accumulating into PSUM partitions
    64j, bank 2i + chunk-parity.
  - Why 64x64 and not 32x32: the Tile framework emits a semaphore
    increment on every matmul (~26 ns, serialized) and walrus emits one
    LDWEIGHTS per matmul (~27 ns, serialized), so a 32x32 schedule
    (3456 MMs) is instruction-overhead-bound at ~39 ns/MM.  At 64x64
    the MM count is 1728 and the 4-way stream rate is 50.6 ns/MM, which
    hides the per-MM overheads.
  - Work unit: batch-pair r x chunk-pair p (18 output rows): 9 taps x
    2 chunks x 4 tiles = 72 matmuls (N = 9*54 = 486 <= one PSUM bank)
    into 4 banks; bank 2i+c holds (batch 2r+i, chunk 2p+c) as
    [128ch, 9, 54].  ppool bufs=2 double-buffers the bank set so
    evictions overlap the next superblock's matmuls.
  - MM emission order: per tap, same-weight chunk-pairs back-to-back
    with consecutive pairs on opposite PE row-groups, so each pair's
    LDWEIGHTS overlaps the other row-group's in-flight matmuls.
  - PSUM fp32 accumulation; evictions alternate scalar/vector engines,
    converting to bf16; DMA out per batch/chunk-pair.
  - bf16 inputs/weights/outputs halve the HBM traffic (memory-regime
    roofline); the host casts x to bf16 and the bf16 output back to
    fp32.  End-to-end rel err ~2.7e-3 (bf16-limited).
"""

import sys

import numpy as np

for _p in ("/opt/trn_rl_repo",):
    if _p not in sys.path:
        sys.path.insert(0, _p)

import ml_dtypes

import concourse.bacc as bacc
import concourse.tile as tile
from concourse import mybir
from concourse.bass_utils import run_bass_kernel_spmd

BF16 = ml_dtypes.bfloat16

N_CORES = 8
B, C, H, W = 16, 1024, 56, 56
KH = KW = 3
HO, WO = H - KH + 1, W - KW + 1  # 54, 54
CPC = C // N_CORES  # 128 channels (64 groups) per core
RPC = 9  # rows per chunk: 9*54 = 486 fp32 <= one PSUM bank
N_CHUNKS = HO // RPC  # 6
N_GROUPS = B // 2  # batch-pairs
N_PAIRS = N_CHUNKS // 2  # chunk-pairs of 18 rows

_NC_CACHE = {}


def _build_program(repeats=1):
    nc = bacc.Bacc(
        "TRN2", target_bir_lowering=False, debug=False, num_devices=N_CORES
    )
    f32 = mybir.dt.float32
    bf16 = mybir.dt.bfloat16

    x_d = nc.declare_dram_parameter("x", [B, CPC, H, W], bf16, isOutput=False)
    wm_d = nc.declare_dram_parameter(
        "wm", [CPC, 2, KH * KW, 64], bf16, isOutput=False
    )
    y_d = nc.declare_dram_parameter("y", [B, CPC, HO, WO], bf16, isOutput=True)

    with tile.TileContext(nc) as tc:
        with (
            tc.tile_pool(name="wpool", bufs=1) as wpool,
            tc.tile_pool(name="xpool", bufs=3) as xpool,
            tc.tile_pool(name="opool", bufs=3) as opool,
            tc.tile_pool(name="ppool", bufs=2, space="PSUM") as ppool,
        ):
            wt = wpool.tile([CPC, 2, KH * KW, 64], bf16)
            nc.sync.dma_start(out=wt[:], in_=wm_d[:])

            # HAM clock-gate warmup on a throwaway psum generation while
            # the first x DMA is in flight.
            pts_w = [
                ppool.tile([CPC, RPC, WO], f32, name=f"pt{b}")
                for b in range(4)
            ]
            for rep in range(8):
                for k in range(4):
                    i, j = k // 2, k % 2
                    nc.tensor.matmul(
                        pts_w[k][64 * j : 64 * j + 64, 0, 0:32],
                        lhsT=wt[64 * i : 64 * i + 64, 0, 0, :],
                        rhs=wt[64 * i : 64 * i + 64, 0, 0:1, 0:32],
                        start=True,
                        stop=True,
                        tile_position=(64 * i, 64 * j),
                    )

            def body():
                _emit_groups(nc, tc, xpool, opool, ppool, x_d, y_d, wt)

            if repeats == 1:
                body()
            else:
                with tc.For_i(0, repeats):
                    body()
    nc.compile()
    return nc


def _emit_groups(nc, tc, xpool, opool, ppool, x_d, y_d, wt):
    f32 = mybir.dt.float32
    bf16 = mybir.dt.bfloat16
    for r in range(N_GROUPS):
        xg = xpool.tile([CPC, 2, H, W], bf16)
        for i in range(2):
            for j in range(2):
                nc.sync.dma_start(
                    out=xg[64 * i : 64 * i + 64, j],
                    in_=x_d[2 * r + i, 64 * j : 64 * j + 64],
                )
        for p in range(N_PAIRS):
            pts = [
                ppool.tile([CPC, RPC, WO], f32, name=f"pt{b}")
                for b in range(4)
            ]
            for t in range(KH * KW):
                kh, kw = t // KW, t % KW
                # Pair-adjacent, row-group-alternating order: each
                # weight's 2 chunk-MMs are back-to-back, and consecutive
                # pairs target opposite PE row-groups so the next pair's
                # LDWEIGHTS pulls ahead under the in-flight matmuls
                # (measured: 107us -> 96us -> 85.6us vs c01-major).
                for i, j in ((0, 0), (1, 0), (0, 1), (1, 1)):
                    for c01 in range(2):
                        r0 = (2 * p + c01) * RPC
                        nc.tensor.matmul(
                            pts[2 * i + c01][64 * j : 64 * j + 64],
                            lhsT=wt[64 * i : 64 * i + 64, j, t, :],
                            rhs=xg[
                                64 * i : 64 * i + 64,
                                j,
                                r0 + kh : r0 + kh + RPC,
                                kw : kw + WO,
                            ],
                            start=(t == 0),
                            stop=(t == KH * KW - 1),
                            tile_position=(64 * i, 64 * j),
                        )
            for i in range(2):
                og = opool.tile([CPC, 2 * RPC, WO], bf16, name=f"og{i}")
                for c01 in range(2):
                    src = pts[2 * i + c01][:]
                    dst = og[:, c01 * RPC : (c01 + 1) * RPC, :]
                    if (i + c01) % 2 == 0:
                        nc.scalar.activation(
                            dst, src, mybir.ActivationFunctionType.Copy
                        )
                    else:
                        nc.vector.tensor_copy(dst, src)
                nc.sync.dma_start(
                    out=y_d[
                        2 * r + i, :, 2 * p * RPC : 2 * (p + 1) * RPC, :
                    ],
                    in_=og[:],
                )


def _get_nc(repeats=1):
    if repeats not in _NC_CACHE:
        _NC_CACHE[repeats] = _build_program(repeats)
    return _NC_CACHE[repeats]


def _make_wm(w, cid):
    """Per-core weight tensor [128, 2, 9, 64] bf16: wm[64*i+a, j, t, oc]
    = w[cid*128+64*j+oc, a%2, kh, kw] when a//2 == oc//2 else 0,
    replicated over row-block i (each row-block needs its own copy for
    LDWEIGHTS)."""
    ws = np.asarray(w[cid * CPC : (cid + 1) * CPC], dtype=np.float32)
    wm = np.zeros((64, 2, KH * KW, 64), dtype=np.float32)
    oc = np.arange(64)
    for j in range(2):
        block = ws[64 * j : 64 * j + 64]  # [64 oc, 2 icg, 3, 3]
        for icg in range(2):
            arow = (oc // 2) * 2 + icg
            wm[arow, j, :, oc] = block[oc, icg].reshape(64, KH * KW)
    return np.tile(wm, (2, 1, 1, 1)).astype(BF16)


def _run(x, w, trace=False, **kwargs):
    nc = _get_nc()
    x_bf = np.asarray(x, dtype=np.float32).astype(BF16)
    in_maps = [
        {
            "x": np.ascontiguousarray(
                x_bf[:, cid * CPC : (cid + 1) * CPC]
            ),
            "wm": _make_wm(w, cid),
        }
        for cid in range(N_CORES)
    ]
    res = run_bass_kernel_spmd(
        nc, in_maps, list(range(N_CORES)), trace=trace, **kwargs
    )
    y = np.concatenate(
        [
            res.results[i]["y"].astype(np.float32)
            for i in range(N_CORES)
        ],
        axis=1,
    )
    return y, res


def kernel(x, w):
    y, _ = _run(x, w, trace=False)
    return y
